# revision 25
# baseline (speedup 1.0000x reference)
"""Trainium2 Bass kernel: spiking multi-head attention (nn_MultiHeadedAttention).

Reference semantics (B=4, T=2048, DIN=100, D=512, h=8 heads, dk=64):
    q = spike(query @ Wq + bq)   (spike = (x >= 1.0) -> {0,1})
    k = spike(key @ Wk + bk);  v = spike(value @ Wv + bv)
    attn = (q @ k^T) * scale, causally masked (keep k<=q), NO softmax
    x = spike(attn @ v)
    x = x.transpose(0,1,3,2).reshape(B,T,h*dk)    # scrambled reshape
    y = spike(x @ Wo + bo)

Key facts exploited:
  * No softmax -> causal attention is LINEAR attention:
        O_t = q_t . M_t  +  intra-block tril(Q K^T) V,   M = sum_j k_j v_j^T
    The running 64x64/head state M accumulates in PSUM across 16 t-blocks,
    so only 16 diagonal 128x128 S-tiles per head are ever materialized.
  * The scrambled reshape maps output rows [256*h, 256*(h+1)) to exactly one
    head h, so head-parallel sharding needs NO cross-core communication.
  * Spiked tensors are {0,1} and S <= 128, M <= 2048 are integers, so fp16
    matmul operands with fp32 PSUM accumulation are bit-exact there.
  * Precision budget (fp32 matmuls cost 4 PE cycles/row, fp16 cost 1):
      - k/v projections run with SINGLE-fp16 operands.  fp16xfp16 products
        are exact in fp32, so the only error is the fp16 rounding of
        key/value/Wk/Wv; a bit-accurate CPU simulation of this exact
        quantization gives rel_err 1.38e-2 (< the 2e-2 gate, deterministic;
        PSUM summation-order noise is ~1e-7 vs ~1e-3 decision gaps).
      - q projection stays fp32 (it is small: K=100) to preserve margin.
      - final projection contracts the exact {0,1} xs against Wo split as
        wo_hi + wo_lo (both fp16, residual ~2^-22) -> bit-accurate.

Sharding: core c -> batch b=c//2, head-group hg=c%2 (4 heads per core).

Hardware pitfalls encoded below:
  * K=64 matmuls whose lhsT sits at partition base 0 vs base 64 execute
    concurrently in disjoint PE row groups; concurrent writes to one PSUM
    bank hang the device.  Even-head (base 0) and odd-head (base 64) K=64
    outputs therefore always target different banks; K=128 matmuls between
    them act as barriers (full row occupancy).
  * start=True zeroes a whole 2KB PSUM bank region, so co-located
    accumulation groups share a single start.
  * DMA-issue instructions cost ~0.6us each on the issuing engine and the
    DMA ring fair-shares bandwidth across in-flight transfers, so loads
    ride few fat transfers (f16 k/v pieces are host-packed so each piece
    is one contiguous 4KB-per-partition transfer) serialized by tiny
    gate-copies into exactly the order compute consumes them.
"""

import os
import numpy as np

B, T, DIN, D = 4, 2048, 100, 512
H, DK = 8, 64
NCORES = 8
HPC = 4          # heads per core
DH = HPC * DK    # 256 projected features per core
P = 128
NT = T // P      # 16 t-blocks
KC = D // P      # 4 contraction chunks of the D=512 dim
NPIECE = 4       # load/pipeline pieces along T (512 t each)

# wpk32 (fp32 pack) column offsets
OFF_WQ32 = 0     # 256 cols, DIN+1 rows (bias row)
OFF_MSK32 = 256  # 256 cols
W32 = 512
# wpk16a (fp16 pack, early) column offsets
OFF_WK = 0       # 4 chunks x 256
OFF_WV = 1024    # 4 chunks x 256
OFF_B16 = 2048   # rows 0-5: bk/bv/bo hi+lo, row 6: ones; 512 cols
W16A = 2560
# wpk16b (fp16 pack, late): per chunk [wo_hi 512 | wo_lo 512]
W16B = 4096

_prog_cache: dict = {}
last_exec_time_ns = None


def _build(scale: float, has_bk: bool, has_bv: bool, has_bo: bool):
    from contextlib import ExitStack

    import concourse.bass as bass
    import concourse.tile as tile
    import concourse.mybir as mybir
    from concourse import bacc
    from concourse.bass import ts
    from concourse import masks

    f32 = mybir.dt.float32
    f16 = mybir.dt.float16
    ALU = mybir.AluOpType
    AF = mybir.ActivationFunctionType
    BIG = float(2 ** 26)

    nc = bacc.Bacc(
        "TRN2", target_bir_lowering=False, debug=False, num_devices=NCORES
    )

    qT = nc.dram_tensor("qT", [P, T], f32, kind="ExternalInput").ap()
    kTp = nc.dram_tensor("kTp", [P, NPIECE * 2048], f16, kind="ExternalInput").ap()
    vTp = nc.dram_tensor("vTp", [P, NPIECE * 2048], f16, kind="ExternalInput").ap()
    wpk32 = nc.dram_tensor("wpk32", [P, W32], f32, kind="ExternalInput").ap()
    wpk16a = nc.dram_tensor("wpk16a", [P, W16A], f16, kind="ExternalInput").ap()
    wpk16b = nc.dram_tensor("wpk16b", [P, W16B], f16, kind="ExternalInput").ap()
    # y logical shape [1024, 512] with row = 256*h + 4*i + m, declared 4D so
    # a final-piece store covers a head PAIR in one full-128-partition DMA
    # (hardware-DGE path; sliced-partition transfers crawl on one engine).
    y = nc.dram_tensor("y", [HPC, 64, 4, D], f32, kind="ExternalOutput").ap()

    with tile.TileContext(nc) as tc, ExitStack() as ctx:
        pool = lambda name, bufs, space="SBUF": ctx.enter_context(
            tc.tile_pool(name=name, bufs=bufs, space=space)
        )
        persist = pool("persist", 1)      # distinct tags -> own slots
        s_pool = pool("s_pool", 4)        # masked S tiles (f16)
        t_pool = pool("t_pool", 4)        # ACT-chain temporaries
        m_pool = pool("m_pool", 2)        # M snapshots
        y_pool = pool("y_pool", 3)        # output staging
        pp = pool("pp", 3, "PSUM")        # projections/final/transposes
        ps = pool("ps", 1, "PSUM")        # S^T tiles (2 parity tags)
        po = pool("po", 2, "PSUM")        # O accumulators
        pm = pool("pm", 1, "PSUM")        # persistent M state

        def ptile(shape, dtype=f32, *, name):
            return persist.tile(shape, dtype, name=name, tag=name)

        # ---- SBUF allocations -----------------------------------------
        qt_sb = ptile([P, T], name="qt_sb")
        kt_sb = ptile([P, NPIECE * 2048], f16, name="kt_sb")
        vt_sb = ptile([P, NPIECE * 2048], f16, name="vt_sb")
        w32_sb = ptile([P, W32], name="w32_sb")
        w16a_sb = ptile([P, W16A], f16, name="w16a_sb")
        w16b_sb = ptile([P, W16B], f16, name="w16b_sb")
        wq_sb = w32_sb[:, OFF_WQ32 : OFF_WQ32 + DH]
        msk_sb = w32_sb[:, OFF_MSK32 : OFF_MSK32 + DH]
        wk16 = [w16a_sb[:, OFF_WK + 256 * c :][:, 0:DH] for c in range(KC)]
        wv16 = [w16a_sb[:, OFF_WV + 256 * c :][:, 0:DH] for c in range(KC)]
        b16 = w16a_sb[:, OFF_B16 : OFF_B16 + 512]
        wo_hi = [w16b_sb[:, 1024 * c :][:, 0:512] for c in range(KC)]
        wo_lo = [w16b_sb[:, 1024 * c + 512 :][:, 0:512] for c in range(KC)]
        idt_sb = ptile([P, P], f16, name="idt_sb")
        # qs/ks: spiked projections, d-major [dk, T]; tile i holds heads
        # 2i (parts 0:64) and 2i+1 (parts 64:128).
        qs = [ptile([P, T], f16, name=f"qs{i}") for i in range(2)]
        ks = [ptile([P, T], f16, name=f"ks{i}") for i in range(2)]
        # vkn: t-major spiked v for all 4 heads (cols 256t+64*hl), f16.
        vkn = ptile([P, DH * NT], f16, name="vkn")
        # kn: t-major spiked k via PE transpose of ks, pair-major:
        # cols 256t + 128*pair + 64*(hl%2)
        kn = ptile([P, DH * NT], f16, name="kn")
        # xs: spiked attention output, laid out xs[p, 1024h + 16d + t_blk]
        # so the final-projection lhsT view has a single stride-16 free dim.
        xs = ptile([P, 1024 * HPC], f16, name="xs")

        # ---- loads ----------------------------------------------------
        # All transfers cover the full 128 partitions (a sliced-partition
        # dst falls into the 1-engine software-DGE path at ~26 GB/s; full
        # transfers fan out across all 16 DMA engines).  Three gated
        # chains run concurrently, each issued from its own engine so a
        # chain's gate-wait never head-of-line-blocks another chain:
        #   W (scalar): w32 -> w16a -> w16b          (weights)
        #   X (sync):   k0  -> v0   -> k1  -> v1     (pieces 0-1)
        #   Y (gpsimd): qT  -> k2   -> v2  -> k3 -> v3
        # A gate (tiny gpsimd copy: read last elem of prev dst, write 1st
        # elem of next dst: RAW + WAW) orders transfers inside a chain in
        # exactly consumption order.
        def gate(nxt, prv):
            nc.gpsimd.tensor_copy(nxt, prv)

        def kpview(pc):
            return kt_sb[:, ts(pc, 2048)], kTp[:, ts(pc, 2048)]

        def vpview(pc):
            return vt_sb[:, ts(pc, 2048)], vTp[:, ts(pc, 2048)]

        def kprobe(pc):
            return kt_sb[0:1, 2048 * pc : 2048 * pc + 1], kt_sb[
                0:1, 2048 * pc + 2047 : 2048 * pc + 2048
            ]

        def vprobe(pc):
            return vt_sb[0:1, 2048 * pc : 2048 * pc + 1], vt_sb[
                0:1, 2048 * pc + 2047 : 2048 * pc + 2048
            ]

        nc.scalar.dma_start(out=w32_sb[:, :], in_=wpk32[:, :])
        nc.sync.dma_start(out=kt_sb[:, ts(0, 2048)], in_=kTp[:, ts(0, 2048)])
        nc.gpsimd.dma_start(out=qt_sb[:, :], in_=qT[:, :])
        # chain W
        gate(w16a_sb[0:1, 0:1], w32_sb[0:1, W32 - 1 : W32])
        nc.scalar.dma_start(out=w16a_sb[:, :], in_=wpk16a[:, :])
        gate(w16b_sb[0:1, 0:1], w16a_sb[0:1, W16A - 1 : W16A])
        nc.scalar.dma_start(out=w16b_sb[:, :], in_=wpk16b[:, :])
        # chain X: k0 -> v0 -> k1 -> v1
        gate(vprobe(0)[0], kprobe(0)[1])
        nc.sync.dma_start(out=vpview(0)[0], in_=vpview(0)[1])
        gate(kprobe(1)[0], vprobe(0)[1])
        nc.sync.dma_start(out=kpview(1)[0], in_=kpview(1)[1])
        gate(vprobe(1)[0], kprobe(1)[1])
        nc.sync.dma_start(out=vpview(1)[0], in_=vpview(1)[1])
        # chain Y: qT -> k2 -> v2 -> k3 -> v3
        gate(kprobe(2)[0], qt_sb[0:1, T - 1 : T])
        nc.gpsimd.dma_start(out=kpview(2)[0], in_=kpview(2)[1])
        gate(vprobe(2)[0], kprobe(2)[1])
        nc.gpsimd.dma_start(out=vpview(2)[0], in_=vpview(2)[1])
        gate(kprobe(3)[0], vprobe(2)[1])
        nc.gpsimd.dma_start(out=kpview(3)[0], in_=kpview(3)[1])
        gate(vprobe(3)[0], kprobe(3)[1])
        nc.gpsimd.dma_start(out=vpview(3)[0], in_=vpview(3)[1])
        masks.make_identity(nc, idt_sb[:, :])

        def spike_act(out_ap, in_ap, nm):
            """out = (in >= 1.0) via two exact Relu ops on the ACT engine."""
            tmp = t_pool.tile(list(out_ap.shape), f32, name=f"tmp_{nm}")
            nc.scalar.activation(tmp[:, :], in_ap, AF.Relu, bias=1.0, scale=-1.0)
            nc.scalar.activation(out_ap, tmp[:, :], AF.Relu, bias=1.0, scale=-BIG)

        # ---- qs projection (fp32; only needs qt) ----------------------
        for half in range(2):
            for ch in range(KC):
                pt = pp.tile([P, 512], f32, name="pt", tag="pt")
                nc.tensor.matmul(
                    pt[:, :],
                    lhsT=wq_sb[: DIN + 1, ts(half, P)],
                    rhs=qt_sb[: DIN + 1, ts(ch, 512)],
                    start=True,
                    stop=True,
                )
                spike_act(qs[half][:, ts(ch, 512)], pt[:, :], "q")

        # ---- pipelined: per piece, ks chunk -> vkn blocks -> attention -
        pm_t = pm.tile([P, DH], f32, name="pm_t")
        xs_r = xs.rearrange(
            "p (he par d t) -> p par he d t", he=2, par=2, d=DK, t=NT
        )

        def ks_chunk(ch):
            for half in range(2):
                pt = pp.tile([P, 512], f32, name="pt", tag="pt")
                for c in range(KC):
                    nc.tensor.matmul(
                        pt[:, :],
                        lhsT=wk16[c][:, ts(half, P)],
                        rhs=kt_sb[:, 2048 * ch + 512 * c :][:, 0:512],
                        start=(c == 0),
                        stop=(c == KC - 1) and not has_bk,
                    )
                if has_bk:
                    nc.tensor.matmul(
                        pt[:, :],
                        lhsT=b16[0:1, ts(half, P)],
                        rhs=b16[6:7, 0:512],
                        start=False,
                        stop=False,
                    )
                    nc.tensor.matmul(
                        pt[:, :],
                        lhsT=b16[1:2, ts(half, P)],
                        rhs=b16[6:7, 0:512],
                        start=False,
                        stop=True,
                    )
                spike_act(ks[half][:, ts(ch, 512)], pt[:, :], "k")
            # t-major spiked K for this chunk's 4 blocks via PE transpose
            # (f16, 1 cycle/row); a [128,128] head-pair tile transpose
            # lands exactly in the pair-major layout the M-update wants.
            for tt in range(4 * ch, 4 * ch + 4):
                for pr in range(2):
                    tp = pp.tile([P, P], f16, name="tp", tag="pt")
                    nc.tensor.transpose(
                        tp[:, :], ks[pr][:, ts(tt, P)], idt_sb[:, :]
                    )
                    nc.vector.tensor_copy(
                        kn[:, DH * tt + P * pr :][:, 0:P], tp[:, :]
                    )

        def vkn_block(tt):
            pt = pp.tile([P, 512], f32, name="pt", tag="pt")
            pc, w = divmod(tt, 4)
            for c in range(KC):
                nc.tensor.matmul(
                    pt[:, 0:DH],
                    lhsT=vt_sb[:, 2048 * pc + 512 * c + P * w :][:, 0:P],
                    rhs=wv16[c][:, :],
                    start=(c == 0),
                    stop=(c == KC - 1) and not has_bv,
                )
            if has_bv:
                nc.tensor.matmul(
                    pt[:, 0:DH],
                    lhsT=b16[6:7, 0:P],
                    rhs=b16[2:3, 0:DH],
                    start=False,
                    stop=False,
                )
                nc.tensor.matmul(
                    pt[:, 0:DH],
                    lhsT=b16[6:7, 0:P],
                    rhs=b16[3:4, 0:DH],
                    start=False,
                    stop=True,
                )
            nc.vector.tensor_scalar(
                vkn[:, ts(tt, DH)], pt[:, 0:DH], 1.0, None, ALU.is_ge
            )

        def attn_block(tt):
            if tt > 0:
                # snapshot M_(<tt); single [128,256] copy covers both
                # partition halves (diagonal 64x64 blocks hold real M)
                m_sb = m_pool.tile([P, DH], f16, name="m_sb")
                nc.scalar.copy(m_sb[:, :], pm_t[:, :])
            else:
                m_sb = None
            s_ps = [
                ps.tile([P, DH], f32, name=f"s_ps{par}", tag=f"s_ps{par}")
                for par in range(2)
            ]
            for hl in range(HPC):
                par, idx = hl % 2, hl // 2
                rows = slice(64 * par, 64 * par + 64)
                nc.tensor.matmul(
                    s_ps[par][:, ts(idx, P)],
                    lhsT=ks[idx][rows, ts(tt, P)],
                    rhs=qs[idx][rows, ts(tt, P)],
                    start=True,
                    stop=True,
                )
            s_sb = [
                s_pool.tile([P, DH], f16, name=f"s_sb{par}", tag=f"s_sb{par}")
                for par in range(2)
            ]
            for par in range(2):
                nc.vector.tensor_tensor(
                    s_sb[par][:, :], s_ps[par][:, :], msk_sb[:, :], op=ALU.mult
                )
            o_ps = [po.tile([P, P], f32, name="o_ps") for _ in range(2)]
            for hl in range(HPC):
                par, idx = hl % 2, hl // 2
                rows = slice(64 * par, 64 * par + 64)
                nc.tensor.matmul(
                    o_ps[par][:, ts(idx, 64)],
                    lhsT=s_sb[par][:, ts(idx, P)],
                    rhs=vkn[:, DH * tt + 64 * hl :][:, 0:64],
                    start=True,
                    stop=(tt == 0),
                )
                if tt > 0:
                    mc = 128 * idx + 64 * par
                    nc.tensor.matmul(
                        o_ps[par][:, ts(idx, 64)],
                        lhsT=qs[idx][rows, ts(tt, P)],
                        rhs=m_sb[rows, mc : mc + 64],
                        start=False,
                        stop=True,
                    )
            # M += K_pair^T V_pair: one K=128,N=128 matmul per head pair;
            # only the diagonal 64x64 blocks are meaningful.  stop=True
            # each block closes the sim's accumulation group so the
            # snapshot read is legal; on HW stop is a no-op and the
            # start=False matmuls keep accumulating.
            for pr in range(2):
                nc.tensor.matmul(
                    pm_t[:, ts(pr, P)],
                    lhsT=kn[:, DH * tt + P * pr :][:, 0:P],
                    rhs=vkn[:, DH * tt + P * pr :][:, 0:P],
                    start=(tt == 0 and pr == 0),
                    stop=(pr == 1),
                    skip_group_check=True,
                )
            # x = spike(scale * O) = (relu(1 - scale*O) <= 0): exact, and
            # splits across the idle ACT/GPSIMD engines.
            for par in range(2):
                xtmp = t_pool.tile([P, P], f32, name="xtmp")
                nc.scalar.activation(
                    xtmp[:, :], o_ps[par][:, :], AF.Relu,
                    bias=1.0, scale=-float(scale),
                )
                nc.vector.tensor_scalar(
                    xs_r[:, par, :, :, tt],
                    xtmp[:, :].rearrange("p (h d) -> p h d", h=2),
                    0.0,
                    None,
                    ALU.is_le,
                )

        def proj_piece(pc):
            ks_chunk(pc)
            for tt in range(4 * pc, 4 * pc + 4):
                vkn_block(tt)

        proj_piece(0)
        proj_piece(1)
        # Final projection runs per piece: output rows r with r%4 == m
        # contract only over attention piece m (X[r, f] =
        # x_att[t=512*(r%4)+f, d=r//4]).  xs col = 16*(64h + r//4) +
        # (4m + cc), so a head PAIR's 128 rows are one stride-16 lhsT.
        xq = xs.rearrange("p (q mc) -> p mc q", q=256, mc=16)

        def final_piece(m):
            for j in range(2):  # head pair: heads 2j, 2j+1
                yp = pp.tile([P, 512], f32, name="pt", tag="pt")
                for cc in range(KC):
                    nc.tensor.matmul(
                        yp[:, :],
                        lhsT=xq[:, 4 * m + cc, ts(j, P)],
                        rhs=wo_hi[cc][:, :],
                        start=(cc == 0),
                        stop=False,
                    )
                    nc.tensor.matmul(
                        yp[:, :],
                        lhsT=xq[:, 4 * m + cc, ts(j, P)],
                        rhs=wo_lo[cc][:, :],
                        start=False,
                        stop=(cc == KC - 1) and not has_bo,
                    )
                if has_bo:
                    nc.tensor.matmul(
                        yp[:, :],
                        lhsT=b16[6:7, 0:P],
                        rhs=b16[4:5, 0:512],
                        start=False,
                        stop=False,
                    )
                    nc.tensor.matmul(
                        yp[:, :],
                        lhsT=b16[6:7, 0:P],
                        rhs=b16[5:6, 0:512],
                        start=False,
                        stop=True,
                    )
                y_sb = y_pool.tile([P, D], f32, name="y_sb")
                nc.vector.tensor_scalar(
                    y_sb[:, :], yp[:, :], 1.0, None, ALU.is_ge
                )
                nc.gpsimd.dma_start(
                    out=y[2 * j : 2 * j + 2, :, m, :], in_=y_sb[:, :]
                )

        for pc in range(4):
            if pc + 2 < 4:
                proj_piece(pc + 2)
            for tt in range(4 * pc, 4 * pc + 4):
                attn_block(tt)
            final_piece(pc)

    nc.compile()
    return nc


def _get_prog(scale, has_bk, has_bv, has_bo):
    key = (scale, has_bk, has_bv, has_bo)
    if key not in _prog_cache:
        _prog_cache[key] = _build(scale, has_bk, has_bv, has_bo)
    return _prog_cache[key]


def _hi_lo(x):
    hi = x.astype(np.float16)
    lo = (x - hi.astype(np.float32)).astype(np.float16)
    return hi, lo


def _pack_piecewise16(at):
    # at: [D, T] fp32 -> f16 packed [128, NPIECE*2048] with
    # out[p, 2048*pc + 512*c + t] = at[128c + p, 512*pc + t]
    a16 = at.astype(np.float16)
    return np.ascontiguousarray(
        a16.reshape(KC, P, NPIECE, 512).transpose(1, 2, 0, 3).reshape(P, -1)
    )


def _pack_weights(Wq, bq, Wk, bk, Wv, bv, Wo, bo, cs):
    w32 = np.zeros((P, W32), np.float32)
    w32[:DIN, OFF_WQ32 : OFF_WQ32 + DH] = Wq[:, cs]
    w32[DIN, OFF_WQ32 : OFF_WQ32 + DH] = bq[cs]
    w32[:, OFF_MSK32 : OFF_MSK32 + DH] = np.tile(
        np.triu(np.ones((P, P), np.float32)), (1, 2)
    )
    w16a = np.zeros((P, W16A), np.float16)
    for c in range(KC):
        w16a[:, OFF_WK + 256 * c : OFF_WK + 256 * (c + 1)] = Wk[
            128 * c : 128 * (c + 1), cs
        ].astype(np.float16)
        w16a[:, OFF_WV + 256 * c : OFF_WV + 256 * (c + 1)] = Wv[
            128 * c : 128 * (c + 1), cs
        ].astype(np.float16)
    bkh, bkl = _hi_lo(bk[cs])
    bvh, bvl = _hi_lo(bv[cs])
    boh, bol = _hi_lo(bo)
    w16a[0, OFF_B16 : OFF_B16 + DH] = bkh
    w16a[1, OFF_B16 : OFF_B16 + DH] = bkl
    w16a[2, OFF_B16 : OFF_B16 + DH] = bvh
    w16a[3, OFF_B16 : OFF_B16 + DH] = bvl
    w16a[4, OFF_B16 : OFF_B16 + D] = boh
    w16a[5, OFF_B16 : OFF_B16 + D] = bol
    w16a[6, OFF_B16 : OFF_B16 + D] = 1.0
    w16b = np.zeros((P, W16B), np.float16)
    for c in range(KC):
        ch, cl = _hi_lo(Wo[128 * c : 128 * (c + 1), :])
        w16b[:, 1024 * c : 1024 * c + 512] = ch
        w16b[:, 1024 * c + 512 : 1024 * c + 1024] = cl
    return w32, w16a, w16b


def kernel(**inputs) -> np.ndarray:
    global last_exec_time_ns
    from concourse.bass_utils import run_bass_kernel_spmd

    g = lambda n: np.asarray(inputs[n], dtype=np.float32)
    query, key, value = g("query"), g("key"), g("value")
    Wq, bq, Wk, bk = g("Wq"), g("bq"), g("Wk"), g("bk")
    Wv, bv, Wo, bo = g("Wv"), g("bv"), g("Wo"), g("bo")
    scale = float(np.asarray(inputs["scale"], dtype=np.float32).reshape(-1)[0])

    has_bk, has_bv, has_bo = (bool(np.any(x)) for x in (bk, bv, bo))
    prog = _get_prog(scale, has_bk, has_bv, has_bo)

    in_maps = []
    for c in range(NCORES):
        b, hg = divmod(c, 2)
        cs = slice(DH * hg, DH * (hg + 1))
        qTa = np.zeros((P, T), np.float32)
        qTa[:DIN] = query[b].T
        qTa[DIN] = 1.0
        w32, w16a, w16b = _pack_weights(Wq, bq, Wk, bk, Wv, bv, Wo, bo, cs)
        in_maps.append(
            {
                "qT": qTa,
                "kTp": _pack_piecewise16(np.ascontiguousarray(key[b].T)),
                "vTp": _pack_piecewise16(np.ascontiguousarray(value[b].T)),
                "wpk32": w32,
                "wpk16a": w16a,
                "wpk16b": w16b,
            }
        )

    trace = os.environ.get("BASS_TRACE", "") not in ("", "0")
    res = run_bass_kernel_spmd(
        prog, in_maps, core_ids=list(range(NCORES)), trace=trace
    )
    last_exec_time_ns = res.exec_time_ns
    if res.exec_time_ns is not None:
        print(f"HW exec time: {res.exec_time_ns} ns")

    out = np.empty((B, T, D), np.float32)
    for c in range(NCORES):
        b, hg = divmod(c, 2)
        out[b, 1024 * hg : 1024 * (hg + 1)] = res.results[c]["y"].reshape(
            1024, D
        )
    return out


# revision 30
# speedup vs baseline: 1.0466x; 1.0466x over previous
"""Trainium2 Bass kernel: spiking multi-head attention (nn_MultiHeadedAttention).

Reference semantics (B=4, T=2048, DIN=100, D=512, h=8 heads, dk=64):
    q = spike(query @ Wq + bq)   (spike = (x >= 1.0) -> {0,1})
    k = spike(key @ Wk + bk);  v = spike(value @ Wv + bv)
    attn = (q @ k^T) * scale, causally masked (keep k<=q), NO softmax
    x = spike(attn @ v)
    x = x.transpose(0,1,3,2).reshape(B,T,h*dk)    # scrambled reshape
    y = spike(x @ Wo + bo)

Key facts exploited:
  * No softmax -> causal attention is LINEAR attention:
        O_t = q_t . M_t  +  intra-block tril(Q K^T) V,   M = sum_j k_j v_j^T
    The running 64x64/head state M accumulates in PSUM across 16 t-blocks,
    so only 16 diagonal 128x128 S-tiles per head are ever materialized.
  * The scrambled reshape maps output rows [256*h, 256*(h+1)) to exactly one
    head h, so head-parallel sharding needs NO cross-core communication.
  * Spiked tensors are {0,1} and S <= 128, M <= 2048 are integers, so fp16
    matmul operands with fp32 PSUM accumulation are bit-exact there.
  * Precision budget (fp32 matmuls cost 4 PE cycles/row, fp16 cost 1):
      - k/v projections run with SINGLE-fp16 operands.  fp16xfp16 products
        are exact in fp32, so the only error is the fp16 rounding of
        key/value/Wk/Wv; a bit-accurate CPU simulation of this exact
        quantization gives rel_err 1.38e-2 (< the 2e-2 gate, deterministic;
        PSUM summation-order noise is ~1e-7 vs ~1e-3 decision gaps).
      - q projection stays fp32 (it is small: K=100) to preserve margin.
      - final projection contracts the exact {0,1} xs against Wo split as
        wo_hi + wo_lo (both fp16, residual ~2^-22) -> bit-accurate.

Sharding: core c -> batch b=c//2, head-group hg=c%2 (4 heads per core).

Hardware pitfalls encoded below:
  * K=64 matmuls whose lhsT sits at partition base 0 vs base 64 execute
    concurrently in disjoint PE row groups; concurrent writes to one PSUM
    bank hang the device.  Even-head (base 0) and odd-head (base 64) K=64
    outputs therefore always target different banks; K=128 matmuls between
    them act as barriers (full row occupancy).
  * start=True zeroes a whole 2KB PSUM bank region, so co-located
    accumulation groups share a single start.
  * DMA-issue instructions cost ~0.6us each on the issuing engine and the
    DMA ring fair-shares bandwidth across in-flight transfers, so loads
    ride few fat transfers (f16 k/v pieces are host-packed so each piece
    is one contiguous 4KB-per-partition transfer) serialized by tiny
    gate-copies into exactly the order compute consumes them.
"""

import os
import numpy as np

B, T, DIN, D = 4, 2048, 100, 512
H, DK = 8, 64
NCORES = 8
HPC = 4          # heads per core
DH = HPC * DK    # 256 projected features per core
P = 128
NT = T // P      # 16 t-blocks
KC = D // P      # 4 contraction chunks of the D=512 dim
NPIECE = 4       # load/pipeline pieces along T (512 t each)

# wpk32 (fp32 pack) column offsets
OFF_WQ32 = 0     # 256 cols, DIN+1 rows (bias row)
OFF_MSK32 = 256  # 256 cols
W32 = 512
# wpk16a (fp16 pack, early) column offsets
OFF_WK = 0       # 4 chunks x 256
OFF_WV = 1024    # 4 chunks x 256
OFF_B16 = 2048   # rows 0-5: bk/bv/bo hi+lo, row 6: ones; 512 cols
W16A = 2560
# wpk16b (fp16 pack, late): per chunk [wo_hi 512 | wo_lo 512]
W16B = 4096

_prog_cache: dict = {}
last_exec_time_ns = None


def _build(scale: float, has_bk: bool, has_bv: bool, has_bo: bool):
    from contextlib import ExitStack

    import concourse.bass as bass
    import concourse.tile as tile
    import concourse.mybir as mybir
    from concourse import bacc
    from concourse.bass import ts
    from concourse import masks

    f32 = mybir.dt.float32
    f16 = mybir.dt.float16
    ALU = mybir.AluOpType
    AF = mybir.ActivationFunctionType
    BIG = float(2 ** 26)

    nc = bacc.Bacc(
        "TRN2", target_bir_lowering=False, debug=False, num_devices=NCORES
    )

    qT = nc.dram_tensor("qT", [P, T], f32, kind="ExternalInput").ap()
    kTp = nc.dram_tensor("kTp", [P, NPIECE * 2048], f16, kind="ExternalInput").ap()
    vTp = nc.dram_tensor("vTp", [P, NPIECE * 2048], f16, kind="ExternalInput").ap()
    wpk32 = nc.dram_tensor("wpk32", [P, W32], f32, kind="ExternalInput").ap()
    wpk16a = nc.dram_tensor("wpk16a", [P, W16A], f16, kind="ExternalInput").ap()
    wpk16b = nc.dram_tensor("wpk16b", [P, W16B], f16, kind="ExternalInput").ap()
    y = nc.dram_tensor("y", [HPC * 256, D], f32, kind="ExternalOutput").ap()

    with tile.TileContext(nc) as tc, ExitStack() as ctx:
        pool = lambda name, bufs, space="SBUF": ctx.enter_context(
            tc.tile_pool(name=name, bufs=bufs, space=space)
        )
        persist = pool("persist", 1)      # distinct tags -> own slots
        s_pool = pool("s_pool", 4)        # masked S tiles (f16)
        t_pool = pool("t_pool", 4)        # ACT-chain temporaries
        m_pool = pool("m_pool", 2)        # M snapshots
        y_pool = pool("y_pool", 3)        # output staging
        pp = pool("pp", 3, "PSUM")        # projections/final/transposes
        ps = pool("ps", 1, "PSUM")        # S^T tiles (2 parity tags)
        po = pool("po", 2, "PSUM")        # O accumulators
        pm = pool("pm", 1, "PSUM")        # persistent M state

        def ptile(shape, dtype=f32, *, name):
            return persist.tile(shape, dtype, name=name, tag=name)

        # ---- SBUF allocations -----------------------------------------
        qt_sb = ptile([P, T], name="qt_sb")
        kt_sb = ptile([P, NPIECE * 2048], f16, name="kt_sb")
        vt_sb = ptile([P, NPIECE * 2048], f16, name="vt_sb")
        w32_sb = ptile([P, W32], name="w32_sb")
        w16a_sb = ptile([P, W16A], f16, name="w16a_sb")
        w16b_sb = ptile([P, W16B], f16, name="w16b_sb")
        wq_sb = w32_sb[:, OFF_WQ32 : OFF_WQ32 + DH]
        msk_sb = w32_sb[:, OFF_MSK32 : OFF_MSK32 + DH]
        wk16 = [w16a_sb[:, OFF_WK + 256 * c :][:, 0:DH] for c in range(KC)]
        wv16 = [w16a_sb[:, OFF_WV + 256 * c :][:, 0:DH] for c in range(KC)]
        b16 = w16a_sb[:, OFF_B16 : OFF_B16 + 512]
        wo_hi = [w16b_sb[:, 1024 * c :][:, 0:512] for c in range(KC)]
        wo_lo = [w16b_sb[:, 1024 * c + 512 :][:, 0:512] for c in range(KC)]
        idt_sb = ptile([P, P], f16, name="idt_sb")
        # qs/ks: spiked projections, d-major [dk, T]; tile i holds heads
        # 2i (parts 0:64) and 2i+1 (parts 64:128).
        qs = [ptile([P, T], f16, name=f"qs{i}") for i in range(2)]
        ks = [ptile([P, T], f16, name=f"ks{i}") for i in range(2)]
        # vkn: t-major spiked v for all 4 heads (cols 256t+64*hl), f16.
        vkn = ptile([P, DH * NT], f16, name="vkn")
        # kn: t-major spiked k via PE transpose of ks, pair-major:
        # cols 256t + 128*pair + 64*(hl%2)
        kn = ptile([P, DH * NT], f16, name="kn")
        # xs: spiked attention output, laid out xs[p, 1024h + 16d + t_blk]
        # so the final-projection lhsT view has a single stride-16 free dim.
        xs = ptile([P, 1024 * HPC], f16, name="xs")

        # ---- loads ----------------------------------------------------
        # All transfers cover the full 128 partitions (a sliced-partition
        # dst falls into the 1-engine software-DGE path at ~26 GB/s; full
        # transfers fan out across all 16 DMA engines).  Three gated
        # chains run concurrently, each issued from its own engine so a
        # chain's gate-wait never head-of-line-blocks another chain:
        #   W (scalar): w32 -> w16a -> w16b          (weights)
        #   X (sync):   k0  -> v0   -> k2  -> v2
        #   Y (gpsimd): qT  -> k1   -> v1  -> k3 -> v3
        # A gate (tiny gpsimd copy: read last elem of prev dst, write 1st
        # elem of next dst: RAW + WAW) orders transfers inside a chain in
        # exactly consumption order.
        def gate(nxt, prv):
            nc.gpsimd.tensor_copy(nxt, prv)

        def kpview(pc):
            return kt_sb[:, ts(pc, 2048)], kTp[:, ts(pc, 2048)]

        def vpview(pc):
            return vt_sb[:, ts(pc, 2048)], vTp[:, ts(pc, 2048)]

        def kprobe(pc):
            return kt_sb[0:1, 2048 * pc : 2048 * pc + 1], kt_sb[
                0:1, 2048 * pc + 2047 : 2048 * pc + 2048
            ]

        def vprobe(pc):
            return vt_sb[0:1, 2048 * pc : 2048 * pc + 1], vt_sb[
                0:1, 2048 * pc + 2047 : 2048 * pc + 2048
            ]

        nc.scalar.dma_start(out=w32_sb[:, :], in_=wpk32[:, :])
        nc.sync.dma_start(out=kt_sb[:, ts(0, 2048)], in_=kTp[:, ts(0, 2048)])
        nc.gpsimd.dma_start(out=qt_sb[:, :], in_=qT[:, :])
        # chain W
        gate(w16a_sb[0:1, 0:1], w32_sb[0:1, W32 - 1 : W32])
        nc.scalar.dma_start(out=w16a_sb[:, :], in_=wpk16a[:, :])
        gate(w16b_sb[0:1, 0:1], w16a_sb[0:1, W16A - 1 : W16A])
        nc.scalar.dma_start(out=w16b_sb[:, :], in_=wpk16b[:, :])
        # chain X: k0 -> v0 -> k2 -> v2
        gate(vprobe(0)[0], kprobe(0)[1])
        nc.sync.dma_start(out=vpview(0)[0], in_=vpview(0)[1])
        gate(kprobe(2)[0], vprobe(0)[1])
        nc.sync.dma_start(out=kpview(2)[0], in_=kpview(2)[1])
        gate(vprobe(2)[0], kprobe(2)[1])
        nc.sync.dma_start(out=vpview(2)[0], in_=vpview(2)[1])
        # chain Y: qT -> k1 -> v1 -> k3 -> v3
        gate(kprobe(1)[0], qt_sb[0:1, T - 1 : T])
        nc.gpsimd.dma_start(out=kpview(1)[0], in_=kpview(1)[1])
        gate(vprobe(1)[0], kprobe(1)[1])
        nc.gpsimd.dma_start(out=vpview(1)[0], in_=vpview(1)[1])
        gate(kprobe(3)[0], vprobe(1)[1])
        nc.gpsimd.dma_start(out=kpview(3)[0], in_=kpview(3)[1])
        gate(vprobe(3)[0], kprobe(3)[1])
        nc.gpsimd.dma_start(out=vpview(3)[0], in_=vpview(3)[1])
        masks.make_identity(nc, idt_sb[:, :])

        def spike_act(out_ap, in_ap, nm):
            """out = (in >= 1.0) via two exact Relu ops on the ACT engine."""
            tmp = t_pool.tile(list(out_ap.shape), f32, name=f"tmp_{nm}")
            nc.scalar.activation(tmp[:, :], in_ap, AF.Relu, bias=1.0, scale=-1.0)
            nc.scalar.activation(out_ap, tmp[:, :], AF.Relu, bias=1.0, scale=-BIG)

        # ---- qs projection (fp32; only needs qt) ----------------------
        for half in range(2):
            for ch in range(KC):
                pt = pp.tile([P, 512], f32, name="pt", tag="pt")
                nc.tensor.matmul(
                    pt[:, :],
                    lhsT=wq_sb[: DIN + 1, ts(half, P)],
                    rhs=qt_sb[: DIN + 1, ts(ch, 512)],
                    start=True,
                    stop=True,
                )
                spike_act(qs[half][:, ts(ch, 512)], pt[:, :], "q")

        # ---- pipelined: per piece, ks chunk -> vkn blocks -> attention -
        pm_t = pm.tile([P, DH], f32, name="pm_t")
        xs_r = xs.rearrange(
            "p (he par d t) -> p par he d t", he=2, par=2, d=DK, t=NT
        )

        def ks_chunk(ch):
            for half in range(2):
                pt = pp.tile([P, 512], f32, name="pt", tag="pt")
                for c in range(KC):
                    nc.tensor.matmul(
                        pt[:, :],
                        lhsT=wk16[c][:, ts(half, P)],
                        rhs=kt_sb[:, 2048 * ch + 512 * c :][:, 0:512],
                        start=(c == 0),
                        stop=(c == KC - 1) and not has_bk,
                    )
                if has_bk:
                    nc.tensor.matmul(
                        pt[:, :],
                        lhsT=b16[0:1, ts(half, P)],
                        rhs=b16[6:7, 0:512],
                        start=False,
                        stop=False,
                    )
                    nc.tensor.matmul(
                        pt[:, :],
                        lhsT=b16[1:2, ts(half, P)],
                        rhs=b16[6:7, 0:512],
                        start=False,
                        stop=True,
                    )
                spike_act(ks[half][:, ts(ch, 512)], pt[:, :], "k")
            # t-major spiked K for this chunk's 4 blocks via PE transpose
            # (f16, 1 cycle/row); a [128,128] head-pair tile transpose
            # lands exactly in the pair-major layout the M-update wants.
            for tt in range(4 * ch, 4 * ch + 4):
                for pr in range(2):
                    tp = pp.tile([P, P], f16, name="tp", tag="pt")
                    nc.tensor.transpose(
                        tp[:, :], ks[pr][:, ts(tt, P)], idt_sb[:, :]
                    )
                    nc.vector.tensor_copy(
                        kn[:, DH * tt + P * pr :][:, 0:P], tp[:, :]
                    )

        def vkn_block(tt):
            pt = pp.tile([P, 512], f32, name="pt", tag="pt")
            pc, w = divmod(tt, 4)
            for c in range(KC):
                nc.tensor.matmul(
                    pt[:, 0:DH],
                    lhsT=vt_sb[:, 2048 * pc + 512 * c + P * w :][:, 0:P],
                    rhs=wv16[c][:, :],
                    start=(c == 0),
                    stop=(c == KC - 1) and not has_bv,
                )
            if has_bv:
                nc.tensor.matmul(
                    pt[:, 0:DH],
                    lhsT=b16[6:7, 0:P],
                    rhs=b16[2:3, 0:DH],
                    start=False,
                    stop=False,
                )
                nc.tensor.matmul(
                    pt[:, 0:DH],
                    lhsT=b16[6:7, 0:P],
                    rhs=b16[3:4, 0:DH],
                    start=False,
                    stop=True,
                )
            nc.vector.tensor_scalar(
                vkn[:, ts(tt, DH)], pt[:, 0:DH], 1.0, None, ALU.is_ge
            )

        def attn_block(tt):
            if tt > 0:
                # snapshot M_(<tt); single [128,256] copy covers both
                # partition halves (diagonal 64x64 blocks hold real M)
                m_sb = m_pool.tile([P, DH], f16, name="m_sb")
                nc.scalar.copy(m_sb[:, :], pm_t[:, :])
            else:
                m_sb = None
            s_ps = [
                ps.tile([P, DH], f32, name=f"s_ps{par}", tag=f"s_ps{par}")
                for par in range(2)
            ]
            for hl in range(HPC):
                par, idx = hl % 2, hl // 2
                rows = slice(64 * par, 64 * par + 64)
                nc.tensor.matmul(
                    s_ps[par][:, ts(idx, P)],
                    lhsT=ks[idx][rows, ts(tt, P)],
                    rhs=qs[idx][rows, ts(tt, P)],
                    start=True,
                    stop=True,
                )
            s_sb = [
                s_pool.tile([P, DH], f16, name=f"s_sb{par}", tag=f"s_sb{par}")
                for par in range(2)
            ]
            for par in range(2):
                nc.vector.tensor_tensor(
                    s_sb[par][:, :], s_ps[par][:, :], msk_sb[:, :], op=ALU.mult
                )
            o_ps = [po.tile([P, P], f32, name="o_ps") for _ in range(2)]
            for hl in range(HPC):
                par, idx = hl % 2, hl // 2
                rows = slice(64 * par, 64 * par + 64)
                nc.tensor.matmul(
                    o_ps[par][:, ts(idx, 64)],
                    lhsT=s_sb[par][:, ts(idx, P)],
                    rhs=vkn[:, DH * tt + 64 * hl :][:, 0:64],
                    start=True,
                    stop=(tt == 0),
                )
                if tt > 0:
                    mc = 128 * idx + 64 * par
                    nc.tensor.matmul(
                        o_ps[par][:, ts(idx, 64)],
                        lhsT=qs[idx][rows, ts(tt, P)],
                        rhs=m_sb[rows, mc : mc + 64],
                        start=False,
                        stop=True,
                    )
            # M += K_pair^T V_pair: one K=128,N=128 matmul per head pair;
            # only the diagonal 64x64 blocks are meaningful.  stop=True
            # each block closes the sim's accumulation group so the
            # snapshot read is legal; on HW stop is a no-op and the
            # start=False matmuls keep accumulating.
            for pr in range(2):
                nc.tensor.matmul(
                    pm_t[:, ts(pr, P)],
                    lhsT=kn[:, DH * tt + P * pr :][:, 0:P],
                    rhs=vkn[:, DH * tt + P * pr :][:, 0:P],
                    start=(tt == 0 and pr == 0),
                    stop=(pr == 1),
                    skip_group_check=True,
                )
            # x = spike(scale * O) = (relu(1 - scale*O) <= 0): exact, and
            # splits across the idle ACT/GPSIMD engines.
            for par in range(2):
                xtmp = t_pool.tile([P, P], f32, name="xtmp")
                nc.scalar.activation(
                    xtmp[:, :], o_ps[par][:, :], AF.Relu,
                    bias=1.0, scale=-float(scale),
                )
                nc.vector.tensor_scalar(
                    xs_r[:, par, :, :, tt],
                    xtmp[:, :].rearrange("p (h d) -> p h d", h=2),
                    0.0,
                    None,
                    ALU.is_le,
                )

        def proj_piece(pc):
            ks_chunk(pc)
            for tt in range(4 * pc, 4 * pc + 4):
                vkn_block(tt)

        proj_piece(0)
        proj_piece(1)
        # Final projection runs per piece: output rows r with r%4 == m
        # contract only over attention piece m (X[r, f] =
        # x_att[t=512*(r%4)+f, d=r//4]).  xs col = 16*(64h + r//4) +
        # (4m + cc), so a head PAIR's 128 rows are one stride-16 lhsT.
        xq = xs.rearrange("p (q mc) -> p mc q", q=256, mc=16)

        def final_piece(m):
            for j in range(2):  # head pair: heads 2j, 2j+1
                yp = pp.tile([P, 512], f32, name="pt", tag="pt")
                for cc in range(KC):
                    nc.tensor.matmul(
                        yp[:, :],
                        lhsT=xq[:, 4 * m + cc, ts(j, P)],
                        rhs=wo_hi[cc][:, :],
                        start=(cc == 0),
                        stop=False,
                    )
                    nc.tensor.matmul(
                        yp[:, :],
                        lhsT=xq[:, 4 * m + cc, ts(j, P)],
                        rhs=wo_lo[cc][:, :],
                        start=False,
                        stop=(cc == KC - 1) and not has_bo,
                    )
                if has_bo:
                    nc.tensor.matmul(
                        yp[:, :],
                        lhsT=b16[6:7, 0:P],
                        rhs=b16[4:5, 0:512],
                        start=False,
                        stop=False,
                    )
                    nc.tensor.matmul(
                        yp[:, :],
                        lhsT=b16[6:7, 0:P],
                        rhs=b16[5:6, 0:512],
                        start=False,
                        stop=True,
                    )
                y_sb = y_pool.tile([P, D], f32, name="y_sb")
                nc.vector.tensor_scalar(
                    y_sb[:, :], yp[:, :], 1.0, None, ALU.is_ge
                )
                for sub in range(2):
                    h = 2 * j + sub
                    nc.gpsimd.dma_start(
                        out=y[256 * h + m : 256 * (h + 1) : 4, :],
                        in_=y_sb[64 * sub : 64 * sub + 64, :],
                    )

        for pc in range(4):
            if pc + 2 < 4:
                proj_piece(pc + 2)
            for tt in range(4 * pc, 4 * pc + 4):
                attn_block(tt)
            final_piece(pc)

    nc.compile()
    return nc


def _get_prog(scale, has_bk, has_bv, has_bo):
    key = (scale, has_bk, has_bv, has_bo)
    if key not in _prog_cache:
        _prog_cache[key] = _build(scale, has_bk, has_bv, has_bo)
    return _prog_cache[key]


def _hi_lo(x):
    hi = x.astype(np.float16)
    lo = (x - hi.astype(np.float32)).astype(np.float16)
    return hi, lo


def _pack_piecewise16(at):
    # at: [D, T] fp32 -> f16 packed [128, NPIECE*2048] with
    # out[p, 2048*pc + 512*c + t] = at[128c + p, 512*pc + t]
    a16 = at.astype(np.float16)
    return np.ascontiguousarray(
        a16.reshape(KC, P, NPIECE, 512).transpose(1, 2, 0, 3).reshape(P, -1)
    )


def _pack_weights(Wq, bq, Wk, bk, Wv, bv, Wo, bo, cs):
    w32 = np.zeros((P, W32), np.float32)
    w32[:DIN, OFF_WQ32 : OFF_WQ32 + DH] = Wq[:, cs]
    w32[DIN, OFF_WQ32 : OFF_WQ32 + DH] = bq[cs]
    w32[:, OFF_MSK32 : OFF_MSK32 + DH] = np.tile(
        np.triu(np.ones((P, P), np.float32)), (1, 2)
    )
    w16a = np.zeros((P, W16A), np.float16)
    for c in range(KC):
        w16a[:, OFF_WK + 256 * c : OFF_WK + 256 * (c + 1)] = Wk[
            128 * c : 128 * (c + 1), cs
        ].astype(np.float16)
        w16a[:, OFF_WV + 256 * c : OFF_WV + 256 * (c + 1)] = Wv[
            128 * c : 128 * (c + 1), cs
        ].astype(np.float16)
    bkh, bkl = _hi_lo(bk[cs])
    bvh, bvl = _hi_lo(bv[cs])
    boh, bol = _hi_lo(bo)
    w16a[0, OFF_B16 : OFF_B16 + DH] = bkh
    w16a[1, OFF_B16 : OFF_B16 + DH] = bkl
    w16a[2, OFF_B16 : OFF_B16 + DH] = bvh
    w16a[3, OFF_B16 : OFF_B16 + DH] = bvl
    w16a[4, OFF_B16 : OFF_B16 + D] = boh
    w16a[5, OFF_B16 : OFF_B16 + D] = bol
    w16a[6, OFF_B16 : OFF_B16 + D] = 1.0
    w16b = np.zeros((P, W16B), np.float16)
    for c in range(KC):
        ch, cl = _hi_lo(Wo[128 * c : 128 * (c + 1), :])
        w16b[:, 1024 * c : 1024 * c + 512] = ch
        w16b[:, 1024 * c + 512 : 1024 * c + 1024] = cl
    return w32, w16a, w16b


def kernel(**inputs) -> np.ndarray:
    global last_exec_time_ns
    from concourse.bass_utils import run_bass_kernel_spmd

    g = lambda n: np.asarray(inputs[n], dtype=np.float32)
    query, key, value = g("query"), g("key"), g("value")
    Wq, bq, Wk, bk = g("Wq"), g("bq"), g("Wk"), g("bk")
    Wv, bv, Wo, bo = g("Wv"), g("bv"), g("Wo"), g("bo")
    scale = float(np.asarray(inputs["scale"], dtype=np.float32).reshape(-1)[0])

    has_bk, has_bv, has_bo = (bool(np.any(x)) for x in (bk, bv, bo))
    prog = _get_prog(scale, has_bk, has_bv, has_bo)

    in_maps = []
    for c in range(NCORES):
        b, hg = divmod(c, 2)
        cs = slice(DH * hg, DH * (hg + 1))
        qTa = np.zeros((P, T), np.float32)
        qTa[:DIN] = query[b].T
        qTa[DIN] = 1.0
        w32, w16a, w16b = _pack_weights(Wq, bq, Wk, bk, Wv, bv, Wo, bo, cs)
        in_maps.append(
            {
                "qT": qTa,
                "kTp": _pack_piecewise16(np.ascontiguousarray(key[b].T)),
                "vTp": _pack_piecewise16(np.ascontiguousarray(value[b].T)),
                "wpk32": w32,
                "wpk16a": w16a,
                "wpk16b": w16b,
            }
        )

    trace = os.environ.get("BASS_TRACE", "") not in ("", "0")
    res = run_bass_kernel_spmd(
        prog, in_maps, core_ids=list(range(NCORES)), trace=trace
    )
    last_exec_time_ns = res.exec_time_ns
    if res.exec_time_ns is not None:
        print(f"HW exec time: {res.exec_time_ns} ns")

    out = np.empty((B, T, D), np.float32)
    for c in range(NCORES):
        b, hg = divmod(c, 2)
        out[b, 1024 * hg : 1024 * (hg + 1)] = res.results[c]["y"]
    return out


# revision 35
# speedup vs baseline: 1.0580x; 1.0109x over previous
"""Trainium2 Bass kernel: spiking multi-head attention (nn_MultiHeadedAttention).

Reference semantics (B=4, T=2048, DIN=100, D=512, h=8 heads, dk=64):
    q = spike(query @ Wq + bq)   (spike = (x >= 1.0) -> {0,1})
    k = spike(key @ Wk + bk);  v = spike(value @ Wv + bv)
    attn = (q @ k^T) * scale, causally masked (keep k<=q), NO softmax
    x = spike(attn @ v)
    x = x.transpose(0,1,3,2).reshape(B,T,h*dk)    # scrambled reshape
    y = spike(x @ Wo + bo)

Key facts exploited:
  * No softmax -> causal attention is LINEAR attention:
        O_t = q_t . M_t  +  intra-block tril(Q K^T) V,   M = sum_j k_j v_j^T
    The running 64x64/head state M accumulates in PSUM across 16 t-blocks,
    so only 16 diagonal 128x128 S-tiles per head are ever materialized.
  * The scrambled reshape maps output rows [256*h, 256*(h+1)) to exactly one
    head h, so head-parallel sharding needs NO cross-core communication.
  * Spiked tensors are {0,1} and S <= 128, M <= 2048 are integers, so fp16
    matmul operands with fp32 PSUM accumulation are bit-exact there.
  * Precision budget (fp32 matmuls cost 4 PE cycles/row, fp16 cost 1):
      - k/v projections run with SINGLE-fp16 operands.  fp16xfp16 products
        are exact in fp32, so the only error is the fp16 rounding of
        key/value/Wk/Wv; a bit-accurate CPU simulation of this exact
        quantization gives rel_err 1.38e-2 (< the 2e-2 gate, deterministic;
        PSUM summation-order noise is ~1e-7 vs ~1e-3 decision gaps).
      - q projection stays fp32 (it is small: K=100) to preserve margin.
      - final projection contracts the exact {0,1} xs against Wo split as
        wo_hi + wo_lo (both fp16, residual ~2^-22) -> bit-accurate.

Sharding: core c -> batch b=c//2, head-group hg=c%2 (4 heads per core).

Hardware pitfalls encoded below:
  * K=64 matmuls whose lhsT sits at partition base 0 vs base 64 execute
    concurrently in disjoint PE row groups; concurrent writes to one PSUM
    bank hang the device.  Even-head (base 0) and odd-head (base 64) K=64
    outputs therefore always target different banks; K=128 matmuls between
    them act as barriers (full row occupancy).
  * start=True zeroes a whole 2KB PSUM bank region, so co-located
    accumulation groups share a single start.
  * DMA-issue instructions cost ~0.6us each on the issuing engine and the
    DMA ring fair-shares bandwidth across in-flight transfers, so loads
    ride few fat transfers (f16 k/v pieces are host-packed so each piece
    is one contiguous 4KB-per-partition transfer) serialized by tiny
    gate-copies into exactly the order compute consumes them.
"""

import os
import numpy as np

B, T, DIN, D = 4, 2048, 100, 512
H, DK = 8, 64
NCORES = 8
HPC = 4          # heads per core
DH = HPC * DK    # 256 projected features per core
P = 128
NT = T // P      # 16 t-blocks
KC = D // P      # 4 contraction chunks of the D=512 dim
NPIECE = 4       # load/pipeline pieces along T (512 t each)

# wpk32 (fp32 pack) column offsets
OFF_WQ32 = 0     # 256 cols, DIN+1 rows (bias row)
OFF_MSK32 = 256  # 256 cols
W32 = 512
# wpk16a (fp16 pack, early) column offsets
OFF_WK = 0       # 4 chunks x 256
OFF_WV = 1024    # 4 chunks x 256
OFF_B16 = 2048   # rows 0-5: bk/bv/bo hi+lo, row 6: ones; 512 cols
W16A = 2560
# wpk16b (fp16 pack, late): per chunk [wo_hi 512 | wo_lo 512]
W16B = 4096

_prog_cache: dict = {}
last_exec_time_ns = None


def _build(scale: float, has_bk: bool, has_bv: bool, has_bo: bool):
    from contextlib import ExitStack

    import concourse.bass as bass
    import concourse.tile as tile
    import concourse.mybir as mybir
    from concourse import bacc
    from concourse.bass import ts
    from concourse import masks

    f32 = mybir.dt.float32
    f16 = mybir.dt.float16
    ALU = mybir.AluOpType
    AF = mybir.ActivationFunctionType
    BIG = float(2 ** 26)

    nc = bacc.Bacc(
        "TRN2", target_bir_lowering=False, debug=False, num_devices=NCORES
    )

    qT = nc.dram_tensor("qT", [P, T], f32, kind="ExternalInput").ap()
    kTp = nc.dram_tensor("kTp", [P, NPIECE * 2048], f16, kind="ExternalInput").ap()
    vTp = nc.dram_tensor("vTp", [P, NPIECE * 2048], f16, kind="ExternalInput").ap()
    wpk32 = nc.dram_tensor("wpk32", [P, W32], f32, kind="ExternalInput").ap()
    wpk16a = nc.dram_tensor("wpk16a", [P, W16A], f16, kind="ExternalInput").ap()
    wpk16b = nc.dram_tensor("wpk16b", [P, W16B], f16, kind="ExternalInput").ap()
    y = nc.dram_tensor("y", [HPC * 256, D], f32, kind="ExternalOutput").ap()

    with tile.TileContext(nc) as tc, ExitStack() as ctx:
        pool = lambda name, bufs, space="SBUF": ctx.enter_context(
            tc.tile_pool(name=name, bufs=bufs, space=space)
        )
        persist = pool("persist", 1)      # distinct tags -> own slots
        s_pool = pool("s_pool", 4)        # masked S tiles (f16)
        t_pool = pool("t_pool", 4)        # ACT-chain temporaries
        m_pool = pool("m_pool", 2)        # M snapshots
        y_pool = pool("y_pool", 3)        # output staging
        pp = pool("pp", 3, "PSUM")        # projections/final/transposes
        ps = pool("ps", 1, "PSUM")        # S^T tiles (2 parity tags)
        po = pool("po", 2, "PSUM")        # O accumulators
        pm = pool("pm", 1, "PSUM")        # persistent M state

        def ptile(shape, dtype=f32, *, name):
            return persist.tile(shape, dtype, name=name, tag=name)

        # ---- SBUF allocations -----------------------------------------
        qt_sb = ptile([P, T], name="qt_sb")
        kt_sb = ptile([P, NPIECE * 2048], f16, name="kt_sb")
        vt_sb = ptile([P, NPIECE * 2048], f16, name="vt_sb")
        w32_sb = ptile([P, W32], name="w32_sb")
        w16a_sb = ptile([P, W16A], f16, name="w16a_sb")
        w16b_sb = ptile([P, W16B], f16, name="w16b_sb")
        wq_sb = w32_sb[:, OFF_WQ32 : OFF_WQ32 + DH]
        msk_sb = w32_sb[:, OFF_MSK32 : OFF_MSK32 + DH]
        wk16 = [w16a_sb[:, OFF_WK + 256 * c :][:, 0:DH] for c in range(KC)]
        wv16 = [w16a_sb[:, OFF_WV + 256 * c :][:, 0:DH] for c in range(KC)]
        b16 = w16a_sb[:, OFF_B16 : OFF_B16 + 512]
        wo_hi = [w16b_sb[:, 1024 * c :][:, 0:512] for c in range(KC)]
        wo_lo = [w16b_sb[:, 1024 * c + 512 :][:, 0:512] for c in range(KC)]
        idt_sb = ptile([P, P], f16, name="idt_sb")
        # qs/ks: spiked projections, d-major [dk, T]; tile i holds heads
        # 2i (parts 0:64) and 2i+1 (parts 64:128).
        qs = [ptile([P, T], f16, name=f"qs{i}") for i in range(2)]
        ks = [ptile([P, T], f16, name=f"ks{i}") for i in range(2)]
        # vkn: t-major spiked v for all 4 heads (cols 256t+64*hl), f16.
        vkn = ptile([P, DH * NT], f16, name="vkn")
        # kn: t-major spiked k via PE transpose of ks, pair-major:
        # cols 256t + 128*pair + 64*(hl%2)
        kn = ptile([P, DH * NT], f16, name="kn")
        # xs: spiked attention output, laid out xs[p, 1024h + 16d + t_blk]
        # so the final-projection lhsT view has a single stride-16 free dim.
        xs = ptile([P, 1024 * HPC], f16, name="xs")

        # ---- loads ----------------------------------------------------
        # All transfers cover the full 128 partitions (a sliced-partition
        # dst falls into the 1-engine software-DGE path at ~26 GB/s; full
        # transfers fan out across all 16 DMA engines).  Three gated
        # chains run concurrently, each issued from its own engine so a
        # chain's gate-wait never head-of-line-blocks another chain:
        #   W (scalar): w32 -> w16a -> w16b          (weights)
        #   X (sync):   k0  -> v0   -> k2  -> v2
        #   Y (gpsimd): qTa -> k1   -> v1  -> k3 -> v3
        #   Z (scalar, ungated): qTb                 (q windows 2-3)
        # A gate (tiny gpsimd copy: read last elem of prev dst, write 1st
        # elem of next dst: RAW + WAW) orders transfers inside a chain in
        # exactly consumption order.
        def gate(nxt, prv):
            nc.gpsimd.tensor_copy(nxt, prv)

        def kpview(pc):
            return kt_sb[:, ts(pc, 2048)], kTp[:, ts(pc, 2048)]

        def vpview(pc):
            return vt_sb[:, ts(pc, 2048)], vTp[:, ts(pc, 2048)]

        def kprobe(pc):
            return kt_sb[0:1, 2048 * pc : 2048 * pc + 1], kt_sb[
                0:1, 2048 * pc + 2047 : 2048 * pc + 2048
            ]

        def vprobe(pc):
            return vt_sb[0:1, 2048 * pc : 2048 * pc + 1], vt_sb[
                0:1, 2048 * pc + 2047 : 2048 * pc + 2048
            ]

        nc.scalar.dma_start(out=w32_sb[:, :], in_=wpk32[:, :])
        nc.sync.dma_start(out=kt_sb[:, ts(0, 2048)], in_=kTp[:, ts(0, 2048)])
        nc.gpsimd.dma_start(out=qt_sb[:, 0:1024], in_=qT[:, 0:1024])
        nc.scalar.dma_start(out=qt_sb[:, 1024:T], in_=qT[:, 1024:T])
        # chain W
        gate(w16a_sb[0:1, 0:1], w32_sb[0:1, W32 - 1 : W32])
        nc.scalar.dma_start(out=w16a_sb[:, :], in_=wpk16a[:, :])
        gate(w16b_sb[0:1, 0:1], w16a_sb[0:1, W16A - 1 : W16A])
        nc.scalar.dma_start(out=w16b_sb[:, :], in_=wpk16b[:, :])
        # chain X: k0 -> v0 -> k2 -> v2
        gate(vprobe(0)[0], kprobe(0)[1])
        nc.sync.dma_start(out=vpview(0)[0], in_=vpview(0)[1])
        gate(kprobe(2)[0], vprobe(0)[1])
        nc.sync.dma_start(out=kpview(2)[0], in_=kpview(2)[1])
        gate(vprobe(2)[0], kprobe(2)[1])
        nc.sync.dma_start(out=vpview(2)[0], in_=vpview(2)[1])
        # chain Y: qTa -> k1 -> v1 -> k3 -> v3
        gate(kprobe(1)[0], qt_sb[0:1, 1023:1024])
        nc.gpsimd.dma_start(out=kpview(1)[0], in_=kpview(1)[1])
        gate(vprobe(1)[0], kprobe(1)[1])
        nc.gpsimd.dma_start(out=vpview(1)[0], in_=vpview(1)[1])
        gate(kprobe(3)[0], vprobe(1)[1])
        nc.gpsimd.dma_start(out=kpview(3)[0], in_=kpview(3)[1])
        gate(vprobe(3)[0], kprobe(3)[1])
        nc.gpsimd.dma_start(out=vpview(3)[0], in_=vpview(3)[1])
        masks.make_identity(nc, idt_sb[:, :])

        def spike_act(out_ap, in_ap, nm):
            """out = (in >= 1.0) via two exact Relu ops on the ACT engine."""
            tmp = t_pool.tile(list(out_ap.shape), f32, name=f"tmp_{nm}")
            nc.scalar.activation(tmp[:, :], in_ap, AF.Relu, bias=1.0, scale=-1.0)
            nc.scalar.activation(out_ap, tmp[:, :], AF.Relu, bias=1.0, scale=-BIG)

        # ---- qs projection (fp32; only needs qt) ----------------------
        for half in range(2):
            for ch in range(KC):
                pt = pp.tile([P, 512], f32, name="pt", tag="pt")
                nc.tensor.matmul(
                    pt[:, :],
                    lhsT=wq_sb[: DIN + 1, ts(half, P)],
                    rhs=qt_sb[: DIN + 1, ts(ch, 512)],
                    start=True,
                    stop=True,
                )
                spike_act(qs[half][:, ts(ch, 512)], pt[:, :], "q")

        # ---- pipelined: per piece, ks chunk -> vkn blocks -> attention -
        pm_t = pm.tile([P, DH], f32, name="pm_t")
        xs_r = xs.rearrange(
            "p (he par d t) -> p par he d t", he=2, par=2, d=DK, t=NT
        )

        def ks_chunk(ch):
            for half in range(2):
                pt = pp.tile([P, 512], f32, name="pt", tag="pt")
                for c in range(KC):
                    nc.tensor.matmul(
                        pt[:, :],
                        lhsT=wk16[c][:, ts(half, P)],
                        rhs=kt_sb[:, 2048 * ch + 512 * c :][:, 0:512],
                        start=(c == 0),
                        stop=(c == KC - 1) and not has_bk,
                    )
                if has_bk:
                    nc.tensor.matmul(
                        pt[:, :],
                        lhsT=b16[0:1, ts(half, P)],
                        rhs=b16[6:7, 0:512],
                        start=False,
                        stop=False,
                    )
                    nc.tensor.matmul(
                        pt[:, :],
                        lhsT=b16[1:2, ts(half, P)],
                        rhs=b16[6:7, 0:512],
                        start=False,
                        stop=True,
                    )
                spike_act(ks[half][:, ts(ch, 512)], pt[:, :], "k")
            # t-major spiked K for this chunk's 4 blocks via PE transpose
            # (f16, 1 cycle/row); a [128,128] head-pair tile transpose
            # lands exactly in the pair-major layout the M-update wants.
            for tt in range(4 * ch, 4 * ch + 4):
                for pr in range(2):
                    tp = pp.tile([P, P], f16, name="tp", tag="pt")
                    nc.tensor.transpose(
                        tp[:, :], ks[pr][:, ts(tt, P)], idt_sb[:, :]
                    )
                    nc.vector.tensor_copy(
                        kn[:, DH * tt + P * pr :][:, 0:P], tp[:, :]
                    )

        def vkn_block(tt):
            pt = pp.tile([P, 512], f32, name="pt", tag="pt")
            pc, w = divmod(tt, 4)
            for c in range(KC):
                nc.tensor.matmul(
                    pt[:, 0:DH],
                    lhsT=vt_sb[:, 2048 * pc + 512 * c + P * w :][:, 0:P],
                    rhs=wv16[c][:, :],
                    start=(c == 0),
                    stop=(c == KC - 1) and not has_bv,
                )
            if has_bv:
                nc.tensor.matmul(
                    pt[:, 0:DH],
                    lhsT=b16[6:7, 0:P],
                    rhs=b16[2:3, 0:DH],
                    start=False,
                    stop=False,
                )
                nc.tensor.matmul(
                    pt[:, 0:DH],
                    lhsT=b16[6:7, 0:P],
                    rhs=b16[3:4, 0:DH],
                    start=False,
                    stop=True,
                )
            nc.vector.tensor_scalar(
                vkn[:, ts(tt, DH)], pt[:, 0:DH], 1.0, None, ALU.is_ge
            )

        def attn_block(tt):
            if tt > 0:
                # snapshot M_(<tt); single [128,256] copy covers both
                # partition halves (diagonal 64x64 blocks hold real M)
                m_sb = m_pool.tile([P, DH], f16, name="m_sb")
                nc.scalar.copy(m_sb[:, :], pm_t[:, :])
            else:
                m_sb = None
            s_ps = [
                ps.tile([P, DH], f32, name=f"s_ps{par}", tag=f"s_ps{par}")
                for par in range(2)
            ]
            for hl in range(HPC):
                par, idx = hl % 2, hl // 2
                rows = slice(64 * par, 64 * par + 64)
                nc.tensor.matmul(
                    s_ps[par][:, ts(idx, P)],
                    lhsT=ks[idx][rows, ts(tt, P)],
                    rhs=qs[idx][rows, ts(tt, P)],
                    start=True,
                    stop=True,
                )
            s_sb = [
                s_pool.tile([P, DH], f16, name=f"s_sb{par}", tag=f"s_sb{par}")
                for par in range(2)
            ]
            for par in range(2):
                nc.vector.tensor_tensor(
                    s_sb[par][:, :], s_ps[par][:, :], msk_sb[:, :], op=ALU.mult
                )
            o_ps = [po.tile([P, P], f32, name="o_ps") for _ in range(2)]
            for hl in range(HPC):
                par, idx = hl % 2, hl // 2
                rows = slice(64 * par, 64 * par + 64)
                nc.tensor.matmul(
                    o_ps[par][:, ts(idx, 64)],
                    lhsT=s_sb[par][:, ts(idx, P)],
                    rhs=vkn[:, DH * tt + 64 * hl :][:, 0:64],
                    start=True,
                    stop=(tt == 0),
                )
                if tt > 0:
                    mc = 128 * idx + 64 * par
                    nc.tensor.matmul(
                        o_ps[par][:, ts(idx, 64)],
                        lhsT=qs[idx][rows, ts(tt, P)],
                        rhs=m_sb[rows, mc : mc + 64],
                        start=False,
                        stop=True,
                    )
            # M += K_pair^T V_pair: one K=128,N=128 matmul per head pair;
            # only the diagonal 64x64 blocks are meaningful.  stop=True
            # each block closes the sim's accumulation group so the
            # snapshot read is legal; on HW stop is a no-op and the
            # start=False matmuls keep accumulating.
            for pr in range(2):
                nc.tensor.matmul(
                    pm_t[:, ts(pr, P)],
                    lhsT=kn[:, DH * tt + P * pr :][:, 0:P],
                    rhs=vkn[:, DH * tt + P * pr :][:, 0:P],
                    start=(tt == 0 and pr == 0),
                    stop=(pr == 1),
                    skip_group_check=True,
                )
            # x = spike(scale * O) = (relu(1 - scale*O) <= 0): exact, and
            # splits across the idle ACT/GPSIMD engines.
            for par in range(2):
                xtmp = t_pool.tile([P, P], f32, name="xtmp")
                nc.scalar.activation(
                    xtmp[:, :], o_ps[par][:, :], AF.Relu,
                    bias=1.0, scale=-float(scale),
                )
                nc.vector.tensor_scalar(
                    xs_r[:, par, :, :, tt],
                    xtmp[:, :].rearrange("p (h d) -> p h d", h=2),
                    0.0,
                    None,
                    ALU.is_le,
                )

        def proj_piece(pc):
            ks_chunk(pc)
            for tt in range(4 * pc, 4 * pc + 4):
                vkn_block(tt)

        proj_piece(0)
        proj_piece(1)
        # Final projection runs per piece: output rows r with r%4 == m
        # contract only over attention piece m (X[r, f] =
        # x_att[t=512*(r%4)+f, d=r//4]).  xs col = 16*(64h + r//4) +
        # (4m + cc), so a head PAIR's 128 rows are one stride-16 lhsT.
        xq = xs.rearrange("p (q mc) -> p mc q", q=256, mc=16)

        def final_piece(m):
            for j in range(2):  # head pair: heads 2j, 2j+1
                yp = pp.tile([P, 512], f32, name="pt", tag="pt")
                for cc in range(KC):
                    nc.tensor.matmul(
                        yp[:, :],
                        lhsT=xq[:, 4 * m + cc, ts(j, P)],
                        rhs=wo_hi[cc][:, :],
                        start=(cc == 0),
                        stop=False,
                    )
                    nc.tensor.matmul(
                        yp[:, :],
                        lhsT=xq[:, 4 * m + cc, ts(j, P)],
                        rhs=wo_lo[cc][:, :],
                        start=False,
                        stop=(cc == KC - 1) and not has_bo,
                    )
                if has_bo:
                    nc.tensor.matmul(
                        yp[:, :],
                        lhsT=b16[6:7, 0:P],
                        rhs=b16[4:5, 0:512],
                        start=False,
                        stop=False,
                    )
                    nc.tensor.matmul(
                        yp[:, :],
                        lhsT=b16[6:7, 0:P],
                        rhs=b16[5:6, 0:512],
                        start=False,
                        stop=True,
                    )
                y_sb = y_pool.tile([P, D], f32, name="y_sb")
                nc.vector.tensor_scalar(
                    y_sb[:, :], yp[:, :], 1.0, None, ALU.is_ge
                )
                for sub in range(2):
                    h = 2 * j + sub
                    nc.gpsimd.dma_start(
                        out=y[256 * h + m : 256 * (h + 1) : 4, :],
                        in_=y_sb[64 * sub : 64 * sub + 64, :],
                    )

        for pc in range(4):
            if pc + 2 < 4:
                proj_piece(pc + 2)
            for tt in range(4 * pc, 4 * pc + 4):
                attn_block(tt)
            final_piece(pc)

    nc.compile()
    return nc


def _get_prog(scale, has_bk, has_bv, has_bo):
    key = (scale, has_bk, has_bv, has_bo)
    if key not in _prog_cache:
        _prog_cache[key] = _build(scale, has_bk, has_bv, has_bo)
    return _prog_cache[key]


def _hi_lo(x):
    hi = x.astype(np.float16)
    lo = (x - hi.astype(np.float32)).astype(np.float16)
    return hi, lo


def _pack_piecewise16(at):
    # at: [D, T] fp32 -> f16 packed [128, NPIECE*2048] with
    # out[p, 2048*pc + 512*c + t] = at[128c + p, 512*pc + t]
    a16 = at.astype(np.float16)
    return np.ascontiguousarray(
        a16.reshape(KC, P, NPIECE, 512).transpose(1, 2, 0, 3).reshape(P, -1)
    )


def _pack_weights(Wq, bq, Wk, bk, Wv, bv, Wo, bo, cs):
    w32 = np.zeros((P, W32), np.float32)
    w32[:DIN, OFF_WQ32 : OFF_WQ32 + DH] = Wq[:, cs]
    w32[DIN, OFF_WQ32 : OFF_WQ32 + DH] = bq[cs]
    w32[:, OFF_MSK32 : OFF_MSK32 + DH] = np.tile(
        np.triu(np.ones((P, P), np.float32)), (1, 2)
    )
    w16a = np.zeros((P, W16A), np.float16)
    for c in range(KC):
        w16a[:, OFF_WK + 256 * c : OFF_WK + 256 * (c + 1)] = Wk[
            128 * c : 128 * (c + 1), cs
        ].astype(np.float16)
        w16a[:, OFF_WV + 256 * c : OFF_WV + 256 * (c + 1)] = Wv[
            128 * c : 128 * (c + 1), cs
        ].astype(np.float16)
    bkh, bkl = _hi_lo(bk[cs])
    bvh, bvl = _hi_lo(bv[cs])
    boh, bol = _hi_lo(bo)
    w16a[0, OFF_B16 : OFF_B16 + DH] = bkh
    w16a[1, OFF_B16 : OFF_B16 + DH] = bkl
    w16a[2, OFF_B16 : OFF_B16 + DH] = bvh
    w16a[3, OFF_B16 : OFF_B16 + DH] = bvl
    w16a[4, OFF_B16 : OFF_B16 + D] = boh
    w16a[5, OFF_B16 : OFF_B16 + D] = bol
    w16a[6, OFF_B16 : OFF_B16 + D] = 1.0
    w16b = np.zeros((P, W16B), np.float16)
    for c in range(KC):
        ch, cl = _hi_lo(Wo[128 * c : 128 * (c + 1), :])
        w16b[:, 1024 * c : 1024 * c + 512] = ch
        w16b[:, 1024 * c + 512 : 1024 * c + 1024] = cl
    return w32, w16a, w16b


def kernel(**inputs) -> np.ndarray:
    global last_exec_time_ns
    from concourse.bass_utils import run_bass_kernel_spmd

    g = lambda n: np.asarray(inputs[n], dtype=np.float32)
    query, key, value = g("query"), g("key"), g("value")
    Wq, bq, Wk, bk = g("Wq"), g("bq"), g("Wk"), g("bk")
    Wv, bv, Wo, bo = g("Wv"), g("bv"), g("Wo"), g("bo")
    scale = float(np.asarray(inputs["scale"], dtype=np.float32).reshape(-1)[0])

    has_bk, has_bv, has_bo = (bool(np.any(x)) for x in (bk, bv, bo))
    prog = _get_prog(scale, has_bk, has_bv, has_bo)

    in_maps = []
    for c in range(NCORES):
        b, hg = divmod(c, 2)
        cs = slice(DH * hg, DH * (hg + 1))
        qTa = np.zeros((P, T), np.float32)
        qTa[:DIN] = query[b].T
        qTa[DIN] = 1.0
        w32, w16a, w16b = _pack_weights(Wq, bq, Wk, bk, Wv, bv, Wo, bo, cs)
        in_maps.append(
            {
                "qT": qTa,
                "kTp": _pack_piecewise16(np.ascontiguousarray(key[b].T)),
                "vTp": _pack_piecewise16(np.ascontiguousarray(value[b].T)),
                "wpk32": w32,
                "wpk16a": w16a,
                "wpk16b": w16b,
            }
        )

    trace = os.environ.get("BASS_TRACE", "") not in ("", "0")
    res = run_bass_kernel_spmd(
        prog, in_maps, core_ids=list(range(NCORES)), trace=trace
    )
    last_exec_time_ns = res.exec_time_ns
    if res.exec_time_ns is not None:
        print(f"HW exec time: {res.exec_time_ns} ns")

    out = np.empty((B, T, D), np.float32)
    for c in range(NCORES):
        b, hg = divmod(c, 2)
        out[b, 1024 * hg : 1024 * (hg + 1)] = res.results[c]["y"]
    return out
